# revision 17
# baseline (speedup 1.0000x reference)
"""Trainium2 Bass kernel for nn_MultiHeadNet (moe_routing).

Network (per row of x [N, 2051]):
  idx = 4*(x0>.5) + 2*(x1>.5) + (x2>.5)            (routing from cols 0..2)
  h   = BN1(relu(BN0(x[:,3:]) @ W_net + b_net))     [N,100]
  g_h = BN2(relu(h @ W1[h'] + b1[h']))  for all 8 heads
  out = g_idx @ W2[idx] + b2[idx]                   [N,128]

Strategy:
  * Data-parallel: 8 NeuronCores x 8192 rows.
  * All BatchNorms are affine (eval mode) -> folded into adjacent GEMM
    weights/biases on the host.
  * Head selection folded into the head-1 GEMM: an extra K=4 block of the
    stationary operand adds BIG*(score-3) to the pre-activation, where
    score = #matching routing bits (linear in the bits).  Mismatched heads
    get <= -BIG before the ReLU -> exact 0.  The per-head output bias is
    applied with a tiny b2^T @ onehot8 matmul (hi+lo bf16 split for accuracy).
  * Host pre-transposes x to feature-major bf16 with 16KB-contiguous
    per-partition runs, so the contraction dim lands on SBUF partitions and
    every DMA is a single 2MB instruction.  Output returns transposed
    [128, R] per core and is flipped on host.
  * All GEMMs in bf16 (PE: 1 cycle/row vs 4 for fp32); routing compares and
    all bias adds stay fp32.
"""

import numpy as np
import ml_dtypes

import concourse.bass as bass
import concourse.mybir as mybir
import concourse.tile as tile
from concourse import bacc
from concourse.bass_utils import run_bass_kernel_spmd

EPS = 1e-5
BIG = 2048.0          # exact in bf16; >> max |pre-activation| (~10)
N_CORES = 8
N_TOTAL = 65536
CHUNK = 512           # rows per PSUM tile (one fp32 bank)
KT = 16               # 2048 / 128 trunk contraction tiles
BF = mybir.dt.bfloat16
F32 = mybir.dt.float32
bf16 = ml_dtypes.bfloat16

_cache = {}


def build(R, reps=1):
    """Build the per-core Bass program for R rows.  Returns nc.

    reps>1 repeats the whole computation (same inputs/outputs) inside one
    NEFF — used only for differential wall-clock timing of the HW body.
    """
    if (R, reps) in _cache:
        return _cache[(R, reps)]
    assert R % CHUNK == 0
    NCH = R // CHUNK

    nc = bacc.Bacc(None, target_bir_lowering=False)
    xq = nc.dram_tensor("xq", [128, NCH, KT, CHUNK], BF, kind="ExternalInput")
    # rows 0..2 = x cols 0..2; row 3 = const 1.0 (so is_gt(.,0.5) yields ones)
    xh = nc.dram_tensor("xh", [4, NCH, CHUNK], F32, kind="ExternalInput")
    wn_d = nc.dram_tensor("wn", [128, KT, 100], BF, kind="ExternalInput")
    w1_d = nc.dram_tensor("w1", [100, 4, 100], BF, kind="ExternalInput")
    am_d = nc.dram_tensor("am", [4, 4, 100], BF, kind="ExternalInput")
    a8_d = nc.dram_tensor("a8", [4, 8], BF, kind="ExternalInput")
    b2_d = nc.dram_tensor("b2m", [8, 2, 128], BF, kind="ExternalInput")
    w2_d = nc.dram_tensor("w2", [100, 4, 128], BF, kind="ExternalInput")
    bn_d = nc.dram_tensor("bnb", [100, 1], F32, kind="ExternalInput")
    b1_d = nc.dram_tensor("b1b", [100, 4], F32, kind="ExternalInput")
    outT = nc.dram_tensor("outT", [128, NCH, CHUNK], F32, kind="ExternalOutput")

    Relu = mybir.ActivationFunctionType.Relu

    with tile.TileContext(nc) as tc:
        with (
            tc.tile_pool(name="singles", bufs=1) as singles,
            tc.tile_pool(name="xpool", bufs=4) as xpool,
            tc.tile_pool(name="xhpool", bufs=4) as xhpool,
            tc.tile_pool(name="bvpool", bufs=4) as bvpool,
            tc.tile_pool(name="hpool", bufs=3) as hpool,
            tc.tile_pool(name="gpool", bufs=2) as gpool,
            tc.tile_pool(name="mpool", bufs=2) as mpool,
            tc.tile_pool(name="opool", bufs=3) as opool,
            tc.tile_pool(name="pst", bufs=2, space="PSUM") as pst,
            tc.tile_pool(name="pss", bufs=2, space="PSUM") as pss,
            tc.tile_pool(name="psh", bufs=2, space="PSUM") as psh,
            tc.tile_pool(name="pso", bufs=2, space="PSUM") as pso,
        ):
            # --- load weights once ---
            wn_s = singles.tile([128, KT, 100], BF)
            nc.sync.dma_start(out=wn_s[:], in_=wn_d[:])
            w1_s = singles.tile([100, 4, 100], BF)
            nc.sync.dma_start(out=w1_s[:], in_=w1_d[:])
            am_s = singles.tile([4, 4, 100], BF)
            nc.sync.dma_start(out=am_s[:], in_=am_d[:])
            a8_s = singles.tile([4, 8], BF)
            nc.sync.dma_start(out=a8_s[:], in_=a8_d[:])
            b2_s = singles.tile([8, 2, 128], BF)
            nc.sync.dma_start(out=b2_s[:], in_=b2_d[:])
            w2_s = singles.tile([100, 4, 128], BF)
            nc.sync.dma_start(out=w2_s[:], in_=w2_d[:])
            bn_s = singles.tile([100, 1], F32)
            nc.sync.dma_start(out=bn_s[:], in_=bn_d[:])
            b1_s = singles.tile([100, 4], F32)
            nc.sync.dma_start(out=b1_s[:], in_=b1_d[:])
            neg5 = singles.tile([8, 1], F32)
            nc.vector.memset(neg5[:], -5.0)

            for c in [c for _ in range(reps) for c in range(NCH)]:
                xt = xpool.tile([128, KT, CHUNK], BF)
                nc.sync.dma_start(out=xt[:], in_=xq[:, c, :, :])
                xh_t = xhpool.tile([4, CHUNK], F32)
                nc.sync.dma_start(out=xh_t[:], in_=xh[:, c, :])

                # routing bit rows [b0,b1,b2,1] as bf16 (exact 0/1; row 3 of
                # xh is a host-supplied 1.0 so the compare yields 1.0 there)
                bv = bvpool.tile([4, CHUNK], BF)
                nc.vector.tensor_scalar(
                    bv[:], xh_t[:], 0.5, None, mybir.AluOpType.is_gt
                )

                # trunk: hT = relu(Wn^T @ xn^T + bn)
                hT = hpool.tile([100, CHUNK], BF)
                ps_t = pst.tile([100, CHUNK], F32)
                for k in range(KT):
                    nc.tensor.matmul(
                        ps_t[:], lhsT=wn_s[:, k, :], rhs=xt[:, k, :],
                        start=(k == 0), stop=(k == KT - 1),
                    )
                nc.scalar.activation(hT[:], ps_t[:], Relu, bias=bn_s[:])

                # one-hot head mask [8, CHUNK]: relu(2*score - 5) in {0,1}
                ps_s = pss.tile([8, CHUNK], F32)
                nc.tensor.matmul(ps_s[:], lhsT=a8_s[:], rhs=bv[:],
                                 start=True, stop=True)
                m8 = mpool.tile([8, CHUNK], BF)
                nc.scalar.activation(m8[:], ps_s[:], Relu, bias=neg5[:], scale=2.0)

                # head-1 (all 8 heads, masked via BIG penalty): g = relu(. + b1)
                g = gpool.tile([100, 4, CHUNK], BF)
                for m in range(4):
                    ps_h = psh.tile([100, CHUNK], F32)
                    nc.tensor.matmul(ps_h[:], lhsT=w1_s[:, m, :], rhs=hT[:],
                                     start=True, stop=False)
                    nc.tensor.matmul(ps_h[:], lhsT=am_s[:, m, :], rhs=bv[:],
                                     start=False, stop=True)
                    nc.scalar.activation(g[:, m, :], ps_h[:], Relu,
                                         bias=b1_s[:, m:m + 1])

                # head-2: out^T = W2stack^T @ g + b2^T @ m8 (hi+lo)
                ps_o = pso.tile([128, CHUNK], F32)
                for m in range(4):
                    nc.tensor.matmul(ps_o[:], lhsT=w2_s[:, m, :], rhs=g[:, m, :],
                                     start=(m == 0), stop=False)
                nc.tensor.matmul(ps_o[:], lhsT=b2_s[:, 0, :], rhs=m8[:],
                                 start=False, stop=False)
                nc.tensor.matmul(ps_o[:], lhsT=b2_s[:, 1, :], rhs=m8[:],
                                 start=False, stop=True)

                ot = opool.tile([128, CHUNK], F32)
                nc.vector.tensor_copy(ot[:], ps_o[:])
                nc.gpsimd.dma_start(out=outT[:, c, :], in_=ot[:])

    nc.finalize()
    _cache[(R, reps)] = nc
    return nc


def _fold_weights(inputs):
    """Fold BN0/BN1/BN2 into GEMM weights; build mask matrices."""
    f32 = np.float32
    s0 = 1.0 / np.sqrt(inputs["bn0_var"].astype(f32) + EPS)
    Wn = (s0[:, None] * inputs["W_net"]).astype(f32)                    # [2048,100]
    bn = (inputs["b_net"] - (inputs["bn0_mean"] * s0) @ inputs["W_net"]).astype(f32)
    s1 = 1.0 / np.sqrt(inputs["bn1_var"].astype(f32) + EPS)
    W1f = (s1[None, :, None] * inputs["W1"]).astype(f32)                # [8,100,50]
    b1f = (inputs["b1"] - (inputs["bn1_mean"] * s1) @ inputs["W1"]).astype(f32)
    s2 = 1.0 / np.sqrt(inputs["bn2_var"].astype(f32) + EPS)             # [8,50]
    W2f = (s2[:, :, None] * inputs["W2"]).astype(f32)                   # [8,50,128]
    b2f = (inputs["b2"] - np.einsum("hd,hdo->ho", inputs["bn2_mean"] * s2,
                                    inputs["W2"])).astype(f32)          # [8,128]

    W1cat = np.concatenate([W1f[h] for h in range(8)], axis=1)          # [100,400]
    b1cat = np.concatenate([b1f[h] for h in range(8)])                  # [400]
    A = np.zeros((4, 400), f32)
    A8 = np.zeros((4, 8), f32)
    for h in range(8):
        a = [(h >> 2) & 1, (h >> 1) & 1, h & 1]
        for k in range(3):
            A[k, h * 50:(h + 1) * 50] = BIG * (2 * a[k] - 1)
            A8[k, h] = 2 * a[k] - 1
        A[3, h * 50:(h + 1) * 50] = -BIG * sum(a)
        A8[3, h] = 3 - sum(a)
    W2stack = np.concatenate([W2f[h] for h in range(8)], axis=0)        # [400,128]
    b2hi = b2f.astype(bf16).astype(f32)
    b2lo = (b2f - b2hi).astype(bf16)

    return {
        "wn": np.ascontiguousarray(
            Wn.astype(bf16).reshape(KT, 128, 100).transpose(1, 0, 2)),   # [128,16,100]
        "w1": np.ascontiguousarray(
            W1cat.astype(bf16).reshape(100, 4, 100)),                    # [100,4,100]
        "am": np.ascontiguousarray(
            A.astype(bf16).reshape(4, 4, 100)),                          # [4,4,100]
        "a8": A8.astype(bf16),
        "b2m": np.ascontiguousarray(
            np.stack([b2hi.astype(bf16), b2lo], axis=1)),                # [8,2,128]
        "w2": np.ascontiguousarray(
            W2stack.astype(bf16).reshape(4, 100, 128).transpose(1, 0, 2)),  # [100,4,128]
        "bnb": np.ascontiguousarray(bn[:, None]),                        # [100,1]
        "b1b": np.ascontiguousarray(b1cat.reshape(4, 100).T),            # [100,4]
    }


def _core_inputs(x_slice, R):
    """Per-core x tensors: xq [128, NCH, 16, 512] bf16, xh [4, NCH, 512] f32."""
    NCH = R // CHUNK
    xq = np.ascontiguousarray(
        x_slice[:, 3:].astype(bf16).reshape(NCH, CHUNK, KT, 128)
        .transpose(3, 0, 2, 1))
    xhv = np.empty((4, NCH, CHUNK), np.float32)
    xhv[0:3] = x_slice[:, 0:3].T.reshape(3, NCH, CHUNK)
    xhv[3] = 1.0
    return {"xq": xq, "xh": xhv}


def run(inputs, R=None, n_cores=N_CORES, **run_kwargs):
    """Run the kernel on the first n_cores*R rows; returns (out, results)."""
    x = np.ascontiguousarray(np.asarray(inputs["x"], np.float32))
    n = x.shape[0] if R is None else n_cores * R
    R = n // n_cores
    nc = build(R)
    w = _fold_weights({k: np.asarray(v, np.float32) for k, v in inputs.items()
                       if k != "x"})
    in_maps = []
    for c in range(n_cores):
        m = dict(w)
        m.update(_core_inputs(x[c * R:(c + 1) * R], R))
        in_maps.append(m)
    res = run_bass_kernel_spmd(nc, in_maps, core_ids=list(range(n_cores)),
                               **run_kwargs)
    out = np.empty((n, 128), np.float32)
    for c in range(n_cores):
        out[c * R:(c + 1) * R] = res.results[c]["outT"].reshape(128, R).T
    return out, res


def kernel(**inputs):
    out, _ = run(inputs)
    return out
